# revision 1
# baseline (speedup 1.0000x reference)
"""Trainium2 Bass kernel for the fused attention+LN+GELU+projection module.

Shapes (hardcoded): x [B=256, S=512, D=512]; k/q/v_w [H=256, D]; attn_bias [S, H];
out_w [D, S*H]; output [B, 1, D].

Distribution across 8 NeuronCores:
 - phases 1-7 (QKV proj, scores, softmax, apply, +bias, LN, GELU): data-parallel
   over batch, 32 batches/core.
 - phase 8 (y = act @ out_w.T): contraction dim S*H sharded 8 ways; AllToAll
   redistributes activations from batch-sharded to contraction-sharded layout;
   each core PE-transposes its received [batch, slice] block and multiplies
   against its 1/8 slice of out_w; AllReduce sums the partial outputs.
"""

import sys

sys.path.insert(0, "/opt/trn_rl_repo")

import numpy as np

import concourse.bacc as bacc
import concourse.tile as tile
from concourse import mybir
from concourse.bass_utils import run_bass_kernel_spmd
from concourse.hw_specs import get_activation_tables
from concourse.tile_rust import add_dep_helper
import bass_rust as _bass_rust

N_CORES = 8
B, S, H, D = 256, 512, 256, 512
NB = B // N_CORES          # batches per core
SCALE = 1.0 / (B ** 0.5)   # score scale (batch-size based, faithful to ref)
LN_EPS = 1e-5
NDT = D // 128             # 4 d-tiles
NST = S // 128             # 4 s-tiles
NHT = H // 128             # 2 h-tiles
SLICE = (S // N_CORES) * H  # 16384 contraction elems per core
NC_T = SLICE // 128        # 128 contraction tiles per core
NBT = (N_CORES * NB) // 128  # global-batch tiles (2)
G = 8                      # ACT-table batch group size

F32 = mybir.dt.float32
F32R = mybir.dt.float32r
AF = mybir.ActivationFunctionType


class _Bacc(bacc.Bacc):
    """Bacc whose activation-table binding is restricted so that exp/ln are
    only servable by natural_log_exp_and_others and gelu by gelu_and_others.
    Avoids per-op ACT_TABLE_LOAD thrash (~2.7us each) from the default
    first-match binding. Table ids keep their act_info.json order."""

    def insert_act_table_loads(self):
        has_activation = any(
            isinstance(i, mybir.InstActivation)
            for b in self.main_func.blocks
            for i in b.instructions
        )
        if not has_activation:
            return
        keep = {"natural_log_exp_and_others", "gelu_and_others"}
        strip = {AF.Exp, AF.Ln, AF.Gelu}
        tables = []
        for name, funcs in get_activation_tables(self.m.arch).items():
            if name not in keep:
                funcs = funcs - strip
            tables.append((name, funcs))
        _bass_rust.insert_act_table_loads(self, tables)


def _build(ln_trivial: bool):
    nc = _Bacc("TRN2", target_bir_lowering=False, debug=False,
               num_devices=N_CORES)

    # ---- DRAM I/O ----
    xT = nc.dram_tensor("xT", [NB, NDT, 128, S], F32R, kind="ExternalInput").ap()
    kq_wT = nc.dram_tensor("kq_wT", [NDT, 128, 2 * H], F32R, kind="ExternalInput").ap()
    v_wT = nc.dram_tensor("v_wT", [NDT, 128, H], F32R, kind="ExternalInput").ap()
    kq_b = nc.dram_tensor("kq_b", [128, 2 * H], F32, kind="ExternalInput").ap()
    v_b2 = nc.dram_tensor("v_b2", [NHT, 128, 1], F32, kind="ExternalInput").ap()
    ab = nc.dram_tensor("ab", [NST, 128, H], F32, kind="ExternalInput").ap()
    outb8 = nc.dram_tensor("outb8", [128, D], F32, kind="ExternalInput").ap()
    ones128 = nc.dram_tensor("ones128", [128, 128], F32R, kind="ExternalInput").ap()
    eye128 = nc.dram_tensor("eye128", [128, 128], F32R, kind="ExternalInput").ap()
    owT = nc.dram_tensor("owT", [NC_T, 128, D], F32R, kind="ExternalInput").ap()
    if not ln_trivial:
        lng = nc.dram_tensor("lng", [128, H], F32, kind="ExternalInput").ap()
        lnb = nc.dram_tensor("lnb", [128, H], F32, kind="ExternalInput").ap()
    y_out = nc.dram_tensor("y", [N_CORES * NB, D], F32, kind="ExternalOutput").ap()

    # internal DRAM (collective bounce buffers)
    a2a_in = nc.dram_tensor("a2a_in", [N_CORES, NB, S // N_CORES, H], F32R).ap()
    a2a_out = nc.dram_tensor("a2a_out", [N_CORES * NB, SLICE], F32R).ap()
    y_bounce = nc.dram_tensor("y_bounce", [N_CORES * NB, D], F32).ap()
    y_red = nc.dram_tensor("y_red", [N_CORES * NB, D], F32).ap()

    with tile.TileContext(nc) as tc:
        with (
            tc.tile_pool(name="const", bufs=1) as constp,
            tc.tile_pool(name="xt", bufs=8) as xtp,
            tc.tile_pool(name="kqsb", bufs=8) as kqp,
            tc.tile_pool(name="vtsb", bufs=4) as vtp,
            tc.tile_pool(name="esb", bufs=4) as ep,
            tc.tile_pool(name="wsb", bufs=4) as wp,
            tc.tile_pool(name="tsb", bufs=40) as tp,
            tc.tile_pool(name="actsb", bufs=8) as actp,
            tc.tile_pool(name="stat", bufs=16) as statp,
            tc.tile_pool(name="lnstat", bufs=40) as lnstatp,
            tc.tile_pool(name="rec", bufs=2) as recp,
        ):
            # ---- persistent constants ----
            kqw_sb = []
            vw_sb = []
            for dt_ in range(NDT):
                t = constp.tile([128, 2 * H], F32R, tag=f"kqw{dt_}")
                nc.sync.dma_start(t[:], kq_wT[dt_])
                kqw_sb.append(t)
                t = constp.tile([128, H], F32R, tag=f"vw{dt_}")
                nc.sync.dma_start(t[:], v_wT[dt_])
                vw_sb.append(t)
            kqb_sb = constp.tile([128, 2 * H], F32, tag="kqb")
            nc.sync.dma_start(kqb_sb[:], kq_b[:])
            vb_sb = []
            for ht in range(NHT):
                t = constp.tile([128, 1], F32, tag=f"vb{ht}")
                nc.sync.dma_start(t[:], v_b2[ht])
                vb_sb.append(t)
            ab_sb = []
            for st in range(NST):
                t = constp.tile([128, H], F32, tag=f"ab{st}")
                nc.sync.dma_start(t[:], ab[st])
                ab_sb.append(t)
            outb_sb = constp.tile([128, D], F32, tag="outb")
            nc.sync.dma_start(outb_sb[:], outb8[:])
            if not ln_trivial:
                lng_sb = constp.tile([128, H], F32, tag="lng")
                nc.sync.dma_start(lng_sb[:], lng[:])
                lnb_sb = constp.tile([128, H], F32, tag="lnb")
                nc.sync.dma_start(lnb_sb[:], lnb[:])
            ones_sb = constp.tile([128, 128], F32R, tag="ones")
            nc.sync.dma_start(ones_sb[:], ones128[:])
            ones_col = ones_sb[:, 0:1]
            ones_row = ones_sb[0:1, :]
            eye_sb = constp.tile([128, 128], F32R, tag="eye")
            nc.sync.dma_start(eye_sb[:], eye128[:])
            eps_sb = constp.tile([128, 1], F32, tag="eps")
            nc.gpsimd.memset(eps_sb[:], LN_EPS)

            # ---- per-batch attention pipeline (ACT-table phase groups) ----
            with (
                tc.tile_pool(name="bigps", bufs=2, space="PSUM") as bigps,
                tc.tile_pool(name="smallps", bufs=4, space="PSUM") as smallps,
            ):
                pend = []            # deferred-GELU state per batch in group
                grp_tbl_insts = []   # this group's exp/ln ACT instructions
                prev_gelu = None     # last gelu instruction of previous group
                for b in range(NB):
                    xt = []
                    for dt_ in range(NDT):
                        t = xtp.tile([128, S], F32R, tag="xt")
                        nc.sync.dma_start(t[:], xT[b, dt_])
                        xt.append(t)

                    # vT[h, s] = sum_d v_wT[d, h] * xT[d, s]  (+v_b per-part)
                    vt_sb = []
                    for ht in range(NHT):
                        ps = bigps.tile([128, S], F32, tag="bigps")
                        for dt_ in range(NDT):
                            nc.tensor.matmul(
                                ps[:], vw_sb[dt_][:, ht * 128:(ht + 1) * 128],
                                xt[dt_][:],
                                start=(dt_ == 0), stop=(dt_ == NDT - 1))
                        t = vtp.tile([128, S], F32R, tag="vt")
                        nc.scalar.activation(t[:], ps[:], AF.Identity,
                                             bias=vb_sb[ht][:])
                        vt_sb.append(t)

                    # kq[s, j] = sum_d x[s, d] * [k_wT | q_wT][d, j]  (+bias)
                    kq_sb = []
                    for st in range(NST):
                        ps = bigps.tile([128, S], F32, tag="bigps")
                        for dt_ in range(NDT):
                            nc.tensor.matmul(
                                ps[:], xt[dt_][:, st * 128:(st + 1) * 128],
                                kqw_sb[dt_][:],
                                start=(dt_ == 0), stop=(dt_ == NDT - 1))
                        t = kqp.tile([128, 2 * H], F32R, tag="kq")
                        nc.vector.tensor_add(t[:], ps[:], kqb_sb[:])
                        kq_sb.append(t)

                    # scores[h, g] = sum_s k[s, h] q[s, g]; e = exp(scores/16)
                    e_sb = []
                    for ht in range(NHT):
                        sc = smallps.tile([128, H], F32, tag="smallps")
                        for st in range(NST):
                            nc.tensor.matmul(
                                sc[:], kq_sb[st][:, ht * 128:(ht + 1) * 128],
                                kq_sb[st][:, H:2 * H],
                                start=(st == 0), stop=(st == NST - 1))
                        t = ep.tile([128, H], F32R, tag="e")
                        ei = nc.scalar.activation(t[:], sc[:], AF.Exp,
                                                  scale=SCALE)
                        grp_tbl_insts.append(ei)
                        e_sb.append(t)

                    # softmax denom over h (partition dim) via ones-matmuls
                    sm = smallps.tile([128, H], F32, tag="smallps")
                    for ht in range(NHT):
                        nc.tensor.matmul(sm[0:1, :], ones_col, e_sb[ht][:],
                                         start=(ht == 0), stop=(ht == NHT - 1))
                    rec_sb = recp.tile([1, H], F32R, tag="rec")
                    with nc.allow_low_precision(reason="softmax recip"):
                        nc.vector.reciprocal(rec_sb[:], sm[0:1, :])
                    bc = smallps.tile([128, H], F32, tag="smallps")
                    nc.tensor.matmul(bc[:], ones_row, rec_sb[:],
                                     start=True, stop=True)
                    w_sb = []
                    for ht in range(NHT):
                        t = wp.tile([128, H], F32R, tag="w")
                        nc.vector.tensor_mul(t[:], e_sb[ht][:], bc[:])
                        w_sb.append(t)

                    # out5[s, g] = sum_h vT[h, s] w[h, g]; +attn_bias; LN stats
                    tl, rl, nl = [], [], []
                    for st in range(NST):
                        p5 = smallps.tile([128, H], F32, tag="smallps")
                        for ht in range(NHT):
                            nc.tensor.matmul(
                                p5[:], vt_sb[ht][:, st * 128:(st + 1) * 128],
                                w_sb[ht][:],
                                start=(ht == 0), stop=(ht == NHT - 1))
                        t_sb = tp.tile([128, H], F32, tag="t")
                        nc.vector.tensor_add(t_sb[:], p5[:], ab_sb[st][:])
                        st6 = statp.tile([128, 6], F32, tag="st6")
                        nc.vector.bn_stats(st6[:], t_sb[:])
                        mv = statp.tile([128, 2], F32, tag="mv")
                        nc.vector.bn_aggr(mv[:], st6[:])
                        # rstd = (var+eps)^-0.5 = exp(-0.5*ln(var+eps)); both
                        # funcs live in the natural_log_exp table set.
                        lnv = lnstatp.tile([128, 1], F32, tag="lnv")
                        li = nc.scalar.activation(lnv[:], mv[:, 1:2], AF.Ln,
                                                  bias=eps_sb[:])
                        grp_tbl_insts.append(li)
                        rstd = lnstatp.tile([128, 1], F32, tag="rstd")
                        ri = nc.scalar.activation(rstd[:], lnv[:], AF.Exp,
                                                  scale=-0.5)
                        grp_tbl_insts.append(ri)
                        nb_t = lnstatp.tile([128, 1], F32, tag="nb")
                        nc.vector.tensor_scalar(nb_t[:], mv[:, 0:1], rstd[:],
                                                -1.0, mybir.AluOpType.mult,
                                                mybir.AluOpType.mult)
                        tl.append(t_sb)
                        rl.append(rstd)
                        nl.append(nb_t)
                    pend.append((b, tl, rl, nl))

                    # ---- deferred GELU pass for the finished group ----
                    if (b + 1) % G == 0:
                        if prev_gelu is not None:
                            # keep ACT table phases disjoint across groups
                            for inst in grp_tbl_insts:
                                add_dep_helper(inst.ins, prev_gelu.ins,
                                               sync=False,
                                               reason="act-table grouping")
                        last_tbl = grp_tbl_insts[-1]
                        grp_tbl_insts = []
                        for pb, tl, rl, nl in pend:
                            for st in range(NST):
                                act_sb = actp.tile([128, H], F32R, tag="act")
                                if ln_trivial:
                                    gi = nc.scalar.activation(
                                        act_sb[:], tl[st][:], AF.Gelu,
                                        bias=nl[st][:], scale=rl[st][:])
                                else:
                                    nrm = tp.tile([128, H], F32, tag="nrm")
                                    nc.scalar.activation(
                                        nrm[:], tl[st][:], AF.Identity,
                                        bias=nl[st][:], scale=rl[st][:])
                                    nc.vector.tensor_mul(nrm[:], nrm[:],
                                                         lng_sb[:])
                                    nc.vector.tensor_add(nrm[:], nrm[:],
                                                         lnb_sb[:])
                                    gi = nc.scalar.activation(
                                        act_sb[:], nrm[:], AF.Gelu)
                                add_dep_helper(gi.ins, last_tbl.ins,
                                               sync=False,
                                               reason="act-table grouping")
                                nc.scalar.dma_start(a2a_in[2 * st, pb],
                                                    act_sb[0:64, :])
                                nc.scalar.dma_start(a2a_in[2 * st + 1, pb],
                                                    act_sb[64:128, :])
                                prev_gelu = gi
                        pend = []

            # ---- redistribute: batch-sharded -> contraction-sharded ----
            nc.gpsimd.collective_compute(
                "AllToAll", mybir.AluOpType.bypass,
                replica_groups=[list(range(N_CORES))],
                ins=[a2a_in.opt()], outs=[a2a_out.opt()])

            # ---- phase 8: y_part[b, d] = sum_sh act[b, sh] * owT[sh, d] ----
            # received a2a_out is [global_b, sh]; PE-transpose 128x128 blocks
            # into [sh, b] stationary tiles, then accumulate over all 128
            # contraction tiles.
            with (
                tc.tile_pool(name="p8L", bufs=4) as p8Lp,
                tc.tile_pool(name="p8a", bufs=4) as p8ap,
                tc.tile_pool(name="p8w", bufs=6) as p8wp,
                tc.tile_pool(name="ysb", bufs=2) as ysbp,
                tc.tile_pool(name="yps", bufs=2, space="PSUM") as yps,
                tc.tile_pool(name="trpps", bufs=4, space="PSUM") as trpps,
            ):
                ypsum = []
                for _bt in range(NBT):
                    yp_t = yps.tile([128, D], F32, tag="yps")
                    ypsum.append(yp_t)
                CW = 2048  # sh columns per L load (8KB per partition line)
                for c8 in range(NC_T * 128 // CW):
                    Ls = []
                    for bt in range(NBT):
                        L = p8Lp.tile([128, CW], F32R, tag=f"L{bt}")
                        nc.scalar.dma_start(
                            L[:], a2a_out[bt * 128:(bt + 1) * 128,
                                          c8 * CW:(c8 + 1) * CW])
                        Ls.append(L)
                    for cc in range(CW // 128):
                        c = c8 * (CW // 128) + cc
                        ow_t = p8wp.tile([128, D], F32R, tag="ow")
                        nc.sync.dma_start(ow_t[:], owT[c])
                        at = p8ap.tile([128, NBT * 128], F32R, tag="at")
                        for bt in range(NBT):
                            trp = trpps.tile([128, 128], F32R, tag="trp")
                            nc.tensor.transpose(
                                trp[:], Ls[bt][:, cc * 128:(cc + 1) * 128],
                                eye_sb[:])
                            nc.vector.tensor_copy(
                                at[:, bt * 128:(bt + 1) * 128], trp[:])
                        for bt in range(NBT):
                            nc.tensor.matmul(
                                ypsum[bt][:], at[:, bt * 128:(bt + 1) * 128],
                                ow_t[:],
                                start=(c == 0), stop=(c == NC_T - 1))
                for bt in range(NBT):
                    y_sb = ysbp.tile([128, D], F32, tag="ysb")
                    nc.vector.tensor_add(y_sb[:], ypsum[bt][:], outb_sb[:])
                    nc.sync.dma_start(y_bounce[bt * 128:(bt + 1) * 128, :],
                                      y_sb[:])

                nc.gpsimd.collective_compute(
                    "AllReduce", mybir.AluOpType.add,
                    replica_groups=[list(range(N_CORES))],
                    ins=[y_bounce.opt()], outs=[y_red.opt()])
                nc.sync.dma_start(y_out[:], y_red[:])

    nc.compile()
    return nc


_CACHE = {}


def _get_program(ln_trivial):
    if ln_trivial not in _CACHE:
        _CACHE[ln_trivial] = _build(ln_trivial)
    return _CACHE[ln_trivial]


def _prep_inputs(x, k_w, k_b, q_w, q_b, v_w, v_b, attn_bias, ln_g, ln_b,
                 out_w, out_b):
    ln_trivial = bool(np.all(ln_g == 1.0) and np.all(ln_b == 0.0))
    kq_wT = np.ascontiguousarray(
        np.concatenate([k_w.T, q_w.T], axis=1)).reshape(NDT, 128, 2 * H)
    v_wT = np.ascontiguousarray(v_w.T).reshape(NDT, 128, H)
    kq_b = np.ascontiguousarray(
        np.tile(np.concatenate([k_b, q_b])[None, :], (128, 1)))
    v_b2 = np.ascontiguousarray(v_b.reshape(NHT, 128, 1))
    ab = np.ascontiguousarray(attn_bias.reshape(NST, 128, H))
    outb8 = np.ascontiguousarray(np.tile((out_b / 8.0)[None, :], (128, 1)))
    owT_full = np.ascontiguousarray(out_w.T)  # [S*H, D]
    shared = dict(kq_wT=kq_wT, v_wT=v_wT, kq_b=kq_b, v_b2=v_b2, ab=ab,
                  outb8=outb8, ones128=np.ones((128, 128), np.float32),
                  eye128=np.eye(128, dtype=np.float32))
    if not ln_trivial:
        shared["lng"] = np.ascontiguousarray(np.tile(ln_g[None, :], (128, 1)))
        shared["lnb"] = np.ascontiguousarray(np.tile(ln_b[None, :], (128, 1)))
    in_maps = []
    for i in range(N_CORES):
        xi = np.ascontiguousarray(
            x[i * NB:(i + 1) * NB].transpose(0, 2, 1)).reshape(NB, NDT, 128, S)
        owi = np.ascontiguousarray(
            owT_full[i * SLICE:(i + 1) * SLICE]).reshape(NC_T, 128, D)
        m = dict(shared)
        m["xT"] = xi
        m["owT"] = owi
        in_maps.append(m)
    return ln_trivial, in_maps


def kernel(**inputs):
    xs = {k: np.asarray(v, dtype=np.float32) for k, v in inputs.items()}
    ln_trivial, in_maps = _prep_inputs(
        xs["x"], xs["k_w"], xs["k_b"], xs["q_w"], xs["q_b"], xs["v_w"],
        xs["v_b"], xs["attn_bias"], xs["ln_g"], xs["ln_b"], xs["out_w"],
        xs["out_b"])
    nc = _get_program(ln_trivial)
    res = run_bass_kernel_spmd(nc, in_maps, core_ids=list(range(N_CORES)))
    y = res.results[0]["y"]  # post-AllReduce: identical on every core
    return y.reshape(B, 1, D).astype(np.float32)

